# revision 7
# baseline (speedup 1.0000x reference)
"""Distributed Trainium2 kernel for the AllocortexSystem retrieval problem.

Reference semantics:
    sparse_prior = where(iso > 0.1, iso, 0)
    mem_norm = max(||row||, 1e-8) per ca3 row
    current = sparse_prior
    5x:
        q_norm = max(||current||, 1e-8)
        sim = (ca3 @ q) / (mem_norm * q_norm)       # cosine sims, in [-1, 1]
        w = softmax(sim)                             # global over all rows
        retrieved = w @ ca3
        current = 0.8 * retrieved + 0.2 * current
    mismatch = mean((iso - current)^2)

Strategy: shard ca3 row-wise over 8 cores. Since sims are cosine similarities
in [-1, 1], softmax needs no max subtraction: each step is ONE streaming pass
over the local shard computing s_partial = sum(exp(sim_i)) and
r_partial = sum(exp(sim_i) * row_i), followed by a tiny [1, 1032] AllReduce.

Per 128-row tile:
  - DVE custom TENSOR_TENSOR_REDUCE: fused (row * q_bcast) multiply +
    free-axis sum -> per-row dots (1/q_norm is pre-folded into q).
  - ACT Exp with per-partition scale = cached 1/mem_norm -> unnormalized w.
  - PE matmul (lhsT = w column [128,1], rhs = tile [128,512]x2) PSUM-accumulated
    across all tiles -> r_partial.
Pass 1 also computes row sumsq via ACT Square+accum_out in the same pass;
1/mem_norm comes from a DVE bitcast-Newton rsqrt (no ACT table switches:
only Exp/Square/Copy are used on ACT, all resident in one table set).
"""

import numpy as np

import concourse.bass as bass
import concourse.bacc as bacc
import concourse.tile as tile
from concourse import mybir
from concourse.bass_utils import run_bass_kernel_spmd
from concourse.dve_ops import TENSOR_TENSOR_REDUCE

f32 = mybir.dt.float32
bf16 = mybir.dt.bfloat16
i32 = mybir.dt.int32
AF = mybir.ActivationFunctionType
ALU = mybir.AluOpType

N_CORES = 8
N_FULL = 262144
D = 1024
P = 128
STEPS = 5
INV_EPS = 1e8          # 1 / EPS, clamp for rsqrt
THR = 0.1
AR_LEN = 1032          # 1024 (r partial) + 1 (s partial) + 7 pad
RSQRT_SEED = 0x5F3759DF


def _rsqrt(nc, y, t1, x, n_iter=3):
    """y = rsqrt(x) elementwise via bit-trick seed + Newton (DVE only).
    y, t1, x: same-shape fp32 APs (t1 scratch). Clamped to INV_EPS so the
    result equals 1/max(sqrt(x), eps) for all x >= 0."""
    nc.vector.tensor_scalar(y.bitcast(i32), x.bitcast(i32),
                            1, None, ALU.arith_shift_right)
    nc.vector.tensor_scalar(y.bitcast(i32), y.bitcast(i32),
                            -1, RSQRT_SEED, ALU.mult, ALU.add)
    for _ in range(n_iter):
        nc.vector.tensor_tensor(t1, y, y, ALU.mult)
        nc.vector.tensor_tensor(t1, t1, x, ALU.mult)
        nc.vector.tensor_scalar(t1, t1, -0.5, 1.5, ALU.mult, ALU.add)
        nc.vector.tensor_tensor(y, y, t1, ALU.mult)
    nc.vector.tensor_scalar_min(y, y, INV_EPS)


def build(n_shard=N_FULL // N_CORES, sup=4, sup_bufs=12):
    """Build + compile the SPMD program for one core's shard of n_shard rows."""
    T = n_shard // P            # 128-row tiles per shard
    nsup = T // sup             # supertiles (DMA granules) per pass
    assert nsup * sup == T

    nc = bacc.Bacc("TRN2", target_bir_lowering=False, debug=False,
                   num_devices=N_CORES)
    iso_d = nc.dram_tensor("isocortex_state", [1, D], f32, kind="ExternalInput")
    ca3_d = nc.dram_tensor("ca3_matrix", [n_shard, D], f32, kind="ExternalInput")
    out_d = nc.dram_tensor("out", [1, D + 1], f32, kind="ExternalOutput")

    rg = [list(range(N_CORES))]

    with tile.TileContext(nc) as tc:
        with (
            tc.tile_pool(name="sup", bufs=sup_bufs) as sup_pool,
            tc.tile_pool(name="singles", bufs=1) as singles,
            tc.tile_pool(name="dummies", bufs=2) as dummies,
            tc.tile_pool(name="stepbuf", bufs=2) as stepbuf,
            tc.tile_pool(name="psum_r", bufs=2, space="PSUM") as psum_r_pool,
            tc.tile_pool(name="psum_m", bufs=1, space="PSUM") as psum_m_pool,
            tc.tile_pool(name="cc", bufs=2, space="DRAM") as cc_pool,
        ):
            # ---- persistent buffers ----
            iso_sb = singles.tile([1, D], f32)
            ones_1x128 = singles.tile([1, P], f32)
            ones_128 = singles.tile([P, 1], f32)
            inv_norms = singles.tile([P, T], f32)   # 1/max(||row||,eps)
            ss = singles.tile([P, T], f32)          # row sumsq (pass 1)
            nrm_t1 = singles.tile([P, sup], f32)    # rsqrt scratch
            ar_in = singles.tile([1, AR_LEN], f32)
            out_sb = singles.tile([1, D + 1], f32)

            nc.sync.dma_start(iso_sb[:], iso_d[:])
            nc.vector.memset(ones_1x128[:], 1.0)
            nc.vector.memset(ones_128[:], 1.0)
            nc.vector.memset(ar_in[:, D:AR_LEN], 0.0)

            # ---- q0 = sparse prior ----
            cur = singles.tile([1, D], f32, name="cur0")
            mask = singles.tile([1, D], f32)
            nc.vector.tensor_scalar(mask[:], iso_sb[:], THR, None, ALU.is_gt)
            nc.vector.tensor_tensor(cur[:], mask[:], iso_sb[:], ALU.mult)

            for step in range(STEPS):
                first = step == 0
                # ---- prologue: qn = cur / max(||cur||, eps), bcast to 128p
                dq = stepbuf.tile([1, D], f32, name="dq")
                ssq = stepbuf.tile([1, 1], f32, name="ssq")
                nc.vector._custom_dve(
                    TENSOR_TENSOR_REDUCE, out=dq[:], in0=cur[:], in1=cur[:],
                    s0=0.0, s1=1.0, accum_out=ssq[:])
                invq = stepbuf.tile([1, 1], f32, name="invq")
                sc_t1 = stepbuf.tile([1, 1], f32, name="sc_t1")
                _rsqrt(nc, invq[:], sc_t1[:], ssq[:])
                qn = stepbuf.tile([1, D], f32, name="qn")
                nc.vector.tensor_scalar(qn[:], cur[:], invq[:], None, ALU.mult)

                psum_b0 = psum_m_pool.tile([P, 512], f32, name="psum_b0")
                psum_b1 = psum_m_pool.tile([P, 512], f32, name="psum_b1")
                nc.tensor.matmul(psum_b0[:], ones_1x128[:], qn[:, 0:512],
                                 start=True, stop=True)
                nc.tensor.matmul(psum_b1[:], ones_1x128[:], qn[:, 512:D],
                                 start=True, stop=True)
                qn_b = stepbuf.tile([P, D], bf16, name="qn_b")
                nc.scalar.copy(qn_b[:, 0:512], psum_b0[:])
                nc.scalar.copy(qn_b[:, 512:D], psum_b1[:])

                # ---- single fused pass over the shard ----
                sims = stepbuf.tile([P, T], f32, name="sims")
                e_buf = stepbuf.tile([P, T], bf16, name="e_buf")
                psum_r0 = psum_r_pool.tile([1, 512], f32, name="psum_r0")
                psum_r1 = psum_r_pool.tile([1, 512], f32, name="psum_r1")

                for s in range(nsup):
                    st = sup_pool.tile([P, sup, D], bf16, name="st")
                    src = ca3_d[s * sup * P:(s + 1) * sup * P, :]
                    nc.gpsimd.dma_start(st[:], src.rearrange("(j p) d -> p j d", p=P))
                    cols = np.s_[:, s * sup:(s + 1) * sup]
                    for j in range(sup):
                        t = s * sup + j
                        if first:
                            # fused dot on DVE (1x) keeps ACT free for Square
                            ttro = dummies.tile([P, D], bf16, name="ttro")
                            nc.vector._custom_dve(
                                TENSOR_TENSOR_REDUCE, out=ttro[:],
                                in0=st[:, j, :], in1=qn_b[:],
                                s0=0.0, s1=1.0, accum_out=sims[:, t:t + 1])
                            sqo = dummies.tile([P, D], bf16, name="sqo")
                            nc.scalar.activation(sqo[:], st[:, j, :], AF.Square,
                                                 accum_out=ss[:, t:t + 1])
                        else:
                            # bf16 multiply on DVE (2x mode) + reduce on ACT
                            ttro = dummies.tile([P, D], bf16, name="ttro")
                            nc.vector.tensor_tensor(ttro[:], st[:, j, :],
                                                    qn_b[:], ALU.mult)
                            nc.scalar.activation(ttro[:], ttro[:], AF.Copy,
                                                 accum_out=sims[:, t:t + 1])
                    if first:
                        _rsqrt(nc, inv_norms[cols], nrm_t1[:], ss[cols])
                    # batched scale + exp for the supertile's 4 sims columns
                    sims_sc = dummies.tile([P, sup], f32, name="sims_sc")
                    nc.vector.tensor_tensor(sims_sc[:], sims[cols],
                                            inv_norms[cols], ALU.mult)
                    nc.scalar.activation(e_buf[cols], sims_sc[:], AF.Exp)
                    for j in range(sup):
                        t = s * sup + j
                        nc.tensor.matmul(psum_r0[:], e_buf[:, t:t + 1],
                                         st[:, j, 0:512],
                                         start=(t == 0), stop=(t == T - 1))
                        nc.tensor.matmul(psum_r1[:], e_buf[:, t:t + 1],
                                         st[:, j, 512:D],
                                         start=(t == 0), stop=(t == T - 1))

                # ---- s_partial = sum(e) ----
                e_rowsum = stepbuf.tile([P, 1], f32, name="e_rowsum")
                nc.vector.tensor_reduce(e_rowsum[:], e_buf[:],
                                        mybir.AxisListType.X, ALU.add)
                psum_s = psum_m_pool.tile([1, 1], f32, name="psum_s")
                nc.tensor.matmul(psum_s[:], e_rowsum[:], ones_128[:],
                                 start=True, stop=True)

                # ---- AllReduce [r_partial | s_partial] ----
                nc.scalar.copy(ar_in[:, 0:512], psum_r0[:])
                nc.scalar.copy(ar_in[:, 512:D], psum_r1[:])
                nc.scalar.copy(ar_in[:, D:D + 1], psum_s[:])
                cc_in = cc_pool.tile([1, AR_LEN], f32, name="cc_in")
                cc_out = cc_pool.tile([1, AR_LEN], f32, name="cc_out",
                                      addr_space="Shared")
                nc.gpsimd.dma_start(cc_in[:], ar_in[:])
                nc.gpsimd.collective_compute(
                    "AllReduce", ALU.add, replica_groups=rg,
                    ins=[cc_in[:].opt()], outs=[cc_out[:].opt()])
                ar_out = stepbuf.tile([1, AR_LEN], f32, name="ar_out")
                nc.gpsimd.dma_start(ar_out[:], cc_out[:])

                # ---- current = 0.8 * (r/s) + 0.2 * current ----
                inv_s = stepbuf.tile([1, 1], f32, name="inv_s")
                nc.vector.reciprocal(inv_s[:], ar_out[:, D:D + 1])
                sc08 = stepbuf.tile([1, 1], f32, name="sc08")
                nc.vector.tensor_scalar_mul(sc08[:], inv_s[:], 0.8)
                ret = stepbuf.tile([1, D], f32, name="ret")
                nc.vector.tensor_scalar(ret[:], ar_out[:, 0:D], sc08[:], None,
                                        ALU.mult)
                cur_next = stepbuf.tile([1, D], f32, name="cur_next")
                nc.vector.tensor_scalar_mul(cur_next[:], cur[:], 0.2)
                nc.vector.tensor_tensor(cur_next[:], cur_next[:], ret[:], ALU.add)
                cur = cur_next

            # ---- mismatch = mean((iso - cur)^2) ----
            diff = singles.tile([1, D], f32)
            nc.vector.tensor_tensor(diff[:], iso_sb[:], cur[:], ALU.subtract)
            dq2 = singles.tile([1, D], f32)
            mm = singles.tile([1, 1], f32)
            nc.vector._custom_dve(
                TENSOR_TENSOR_REDUCE, out=dq2[:], in0=diff[:], in1=diff[:],
                s0=0.0, s1=1.0 / D, accum_out=mm[:])

            nc.scalar.copy(out_sb[:, 0:D], cur[:])
            nc.scalar.copy(out_sb[:, D:D + 1], mm[:])
            nc.sync.dma_start(out_d[:], out_sb[:])

    nc.compile()
    return nc


_cache = {}


def _get_nc(n_shard):
    if n_shard not in _cache:
        _cache[n_shard] = build(n_shard)
    return _cache[n_shard]


def kernel(isocortex_state, ca3_matrix):
    iso = np.ascontiguousarray(np.asarray(isocortex_state, dtype=np.float32))
    ca3 = np.asarray(ca3_matrix, dtype=np.float32)
    n = ca3.shape[0]
    n_shard = n // N_CORES
    nc = _get_nc(n_shard)
    shards = ca3.reshape(N_CORES, n_shard, D)
    in_maps = [
        {"isocortex_state": iso, "ca3_matrix": np.ascontiguousarray(shards[i])}
        for i in range(N_CORES)
    ]
    r = run_bass_kernel_spmd(nc, in_maps, core_ids=list(range(N_CORES)))
    out = r.results[0]["out"]
    current = np.array(out[:, 0:D], dtype=np.float32)
    mismatch = np.array(out[0, D], dtype=np.float32)
    return current, mismatch


# revision 8
# speedup vs baseline: 1.0584x; 1.0584x over previous
"""Distributed Trainium2 kernel for the AllocortexSystem retrieval problem.

Reference semantics:
    sparse_prior = where(iso > 0.1, iso, 0)
    mem_norm = max(||row||, 1e-8) per ca3 row
    current = sparse_prior
    5x:
        q_norm = max(||current||, 1e-8)
        sim = (ca3 @ q) / (mem_norm * q_norm)       # cosine sims, in [-1, 1]
        w = softmax(sim)                             # global over all rows
        retrieved = w @ ca3
        current = 0.8 * retrieved + 0.2 * current
    mismatch = mean((iso - current)^2)

Strategy: shard ca3 row-wise over 8 cores. Since sims are cosine similarities
in [-1, 1], softmax needs no max subtraction: each step is ONE streaming pass
over the local shard computing s_partial = sum(exp(sim_i)) and
r_partial = sum(exp(sim_i) * row_i), followed by a tiny [1, 1032] AllReduce.

Per 128-row tile:
  - DVE custom TENSOR_TENSOR_REDUCE: fused (row * q_bcast) multiply +
    free-axis sum -> per-row dots (1/q_norm is pre-folded into q).
  - ACT Exp with per-partition scale = cached 1/mem_norm -> unnormalized w.
  - PE matmul (lhsT = w column [128,1], rhs = tile [128,512]x2) PSUM-accumulated
    across all tiles -> r_partial.
Pass 1 also computes row sumsq via ACT Square+accum_out in the same pass;
1/mem_norm comes from a DVE bitcast-Newton rsqrt (no ACT table switches:
only Exp/Square/Copy are used on ACT, all resident in one table set).
"""

import numpy as np

import concourse.bass as bass
import concourse.bacc as bacc
import concourse.tile as tile
from concourse import mybir
from concourse.bass_utils import run_bass_kernel_spmd
from concourse.dve_ops import TENSOR_TENSOR_REDUCE

f32 = mybir.dt.float32
bf16 = mybir.dt.bfloat16
i32 = mybir.dt.int32
AF = mybir.ActivationFunctionType
ALU = mybir.AluOpType

N_CORES = 8
N_FULL = 262144
D = 1024
P = 128
STEPS = 5
INV_EPS = 1e8          # 1 / EPS, clamp for rsqrt
THR = 0.1
AR_LEN = 1032          # 1024 (r partial) + 1 (s partial) + 7 pad
RSQRT_SEED = 0x5F3759DF


def _rsqrt(nc, y, t1, x, n_iter=3):
    """y = rsqrt(x) elementwise via bit-trick seed + Newton (DVE only).
    y, t1, x: same-shape fp32 APs (t1 scratch). Clamped to INV_EPS so the
    result equals 1/max(sqrt(x), eps) for all x >= 0."""
    nc.vector.tensor_scalar(y.bitcast(i32), x.bitcast(i32),
                            1, None, ALU.arith_shift_right)
    nc.vector.tensor_scalar(y.bitcast(i32), y.bitcast(i32),
                            -1, RSQRT_SEED, ALU.mult, ALU.add)
    for _ in range(n_iter):
        nc.vector.tensor_tensor(t1, y, y, ALU.mult)
        nc.vector.tensor_tensor(t1, t1, x, ALU.mult)
        nc.vector.tensor_scalar(t1, t1, -0.5, 1.5, ALU.mult, ALU.add)
        nc.vector.tensor_tensor(y, y, t1, ALU.mult)
    nc.vector.tensor_scalar_min(y, y, INV_EPS)


def build(n_shard=N_FULL // N_CORES, sup=4, sup_bufs=12):
    """Build + compile the SPMD program for one core's shard of n_shard rows."""
    T = n_shard // P            # 128-row tiles per shard
    nsup = T // sup             # supertiles (DMA granules) per pass
    assert nsup * sup == T

    nc = bacc.Bacc("TRN2", target_bir_lowering=False, debug=False,
                   num_devices=N_CORES)
    iso_d = nc.dram_tensor("isocortex_state", [1, D], f32, kind="ExternalInput")
    ca3_d = nc.dram_tensor("ca3_matrix", [n_shard, D], f32, kind="ExternalInput")
    out_d = nc.dram_tensor("out", [1, D + 1], f32, kind="ExternalOutput")

    rg = [list(range(N_CORES))]

    with tile.TileContext(nc) as tc:
        with (
            tc.tile_pool(name="sup", bufs=sup_bufs) as sup_pool,
            tc.tile_pool(name="singles", bufs=1) as singles,
            tc.tile_pool(name="dummies", bufs=2) as dummies,
            tc.tile_pool(name="stepbuf", bufs=2) as stepbuf,
            tc.tile_pool(name="psum_r", bufs=2, space="PSUM") as psum_r_pool,
            tc.tile_pool(name="psum_m", bufs=1, space="PSUM") as psum_m_pool,
            tc.tile_pool(name="cc", bufs=2, space="DRAM") as cc_pool,
        ):
            # ---- persistent buffers ----
            iso_sb = singles.tile([1, D], f32)
            ones_1x128 = singles.tile([1, P], f32)
            ones_128 = singles.tile([P, 1], f32)
            inv_norms = singles.tile([P, T], f32)   # 1/max(||row||,eps)
            ss = singles.tile([P, T], f32)          # row sumsq (pass 1)
            nrm_t1 = singles.tile([P, sup], f32)    # rsqrt scratch
            ar_in = singles.tile([1, AR_LEN], f32)
            out_sb = singles.tile([1, D + 1], f32)

            nc.sync.dma_start(iso_sb[:], iso_d[:])
            nc.vector.memset(ones_1x128[:], 1.0)
            nc.vector.memset(ones_128[:], 1.0)
            nc.vector.memset(ar_in[:, D:AR_LEN], 0.0)

            # ---- q0 = sparse prior ----
            cur = singles.tile([1, D], f32, name="cur0")
            mask = singles.tile([1, D], f32)
            nc.vector.tensor_scalar(mask[:], iso_sb[:], THR, None, ALU.is_gt)
            nc.vector.tensor_tensor(cur[:], mask[:], iso_sb[:], ALU.mult)

            for step in range(STEPS):
                first = step == 0
                # ---- prologue: qn = cur / max(||cur||, eps), bcast to 128p
                dq = stepbuf.tile([1, D], f32, name="dq")
                ssq = stepbuf.tile([1, 1], f32, name="ssq")
                nc.vector._custom_dve(
                    TENSOR_TENSOR_REDUCE, out=dq[:], in0=cur[:], in1=cur[:],
                    s0=0.0, s1=1.0, accum_out=ssq[:])
                invq = stepbuf.tile([1, 1], f32, name="invq")
                sc_t1 = stepbuf.tile([1, 1], f32, name="sc_t1")
                _rsqrt(nc, invq[:], sc_t1[:], ssq[:])
                qn = stepbuf.tile([1, D], f32, name="qn")
                nc.vector.tensor_scalar(qn[:], cur[:], invq[:], None, ALU.mult)

                psum_b0 = psum_m_pool.tile([P, 512], f32, name="psum_b0")
                psum_b1 = psum_m_pool.tile([P, 512], f32, name="psum_b1")
                nc.tensor.matmul(psum_b0[:], ones_1x128[:], qn[:, 0:512],
                                 start=True, stop=True)
                nc.tensor.matmul(psum_b1[:], ones_1x128[:], qn[:, 512:D],
                                 start=True, stop=True)
                qn_b = stepbuf.tile([P, D], bf16, name="qn_b")
                nc.scalar.copy(qn_b[:, 0:512], psum_b0[:])
                nc.scalar.copy(qn_b[:, 512:D], psum_b1[:])

                # ---- single fused pass over the shard ----
                sims = stepbuf.tile([P, T], f32, name="sims")
                e_buf = stepbuf.tile([P, T], bf16, name="e_buf")
                psum_r0 = psum_r_pool.tile([1, 512], f32, name="psum_r0")
                psum_r1 = psum_r_pool.tile([1, 512], f32, name="psum_r1")

                for s in range(nsup):
                    st = sup_pool.tile([P, sup, D], bf16, name="st")
                    src = ca3_d[s * sup * P:(s + 1) * sup * P, :]
                    nc.gpsimd.dma_start(st[:], src.rearrange("(j p) d -> p j d", p=P))
                    cols = np.s_[:, s * sup:(s + 1) * sup]
                    for j in range(sup):
                        t = s * sup + j
                        if first:
                            # fused dot on DVE (1x) keeps ACT free for Square
                            ttro = dummies.tile([P, D], bf16, name="ttro")
                            nc.vector._custom_dve(
                                TENSOR_TENSOR_REDUCE, out=ttro[:],
                                in0=st[:, j, :], in1=qn_b[:],
                                s0=0.0, s1=1.0, accum_out=sims[:, t:t + 1])
                            sqo = dummies.tile([P, D], bf16, name="sqo")
                            nc.scalar.activation(sqo[:], st[:, j, :], AF.Square,
                                                 accum_out=ss[:, t:t + 1])
                        else:
                            ttro = dummies.tile([P, D], bf16, name="ttro")
                            nc.vector._custom_dve(
                                TENSOR_TENSOR_REDUCE, out=ttro[:],
                                in0=st[:, j, :], in1=qn_b[:],
                                s0=0.0, s1=1.0, accum_out=sims[:, t:t + 1])
                    if first:
                        _rsqrt(nc, inv_norms[cols], nrm_t1[:], ss[cols])
                    # batched scale + exp for the supertile's 4 sims columns
                    sims_sc = dummies.tile([P, sup], f32, name="sims_sc")
                    nc.vector.tensor_tensor(sims_sc[:], sims[cols],
                                            inv_norms[cols], ALU.mult)
                    nc.scalar.activation(e_buf[cols], sims_sc[:], AF.Exp)
                    for j in range(sup):
                        t = s * sup + j
                        nc.tensor.matmul(psum_r0[:], e_buf[:, t:t + 1],
                                         st[:, j, 0:512],
                                         start=(t == 0), stop=(t == T - 1))
                        nc.tensor.matmul(psum_r1[:], e_buf[:, t:t + 1],
                                         st[:, j, 512:D],
                                         start=(t == 0), stop=(t == T - 1))

                # ---- s_partial = sum(e) ----
                e_rowsum = stepbuf.tile([P, 1], f32, name="e_rowsum")
                nc.vector.tensor_reduce(e_rowsum[:], e_buf[:],
                                        mybir.AxisListType.X, ALU.add)
                psum_s = psum_m_pool.tile([1, 1], f32, name="psum_s")
                nc.tensor.matmul(psum_s[:], e_rowsum[:], ones_128[:],
                                 start=True, stop=True)

                # ---- AllReduce [r_partial | s_partial] ----
                nc.scalar.copy(ar_in[:, 0:512], psum_r0[:])
                nc.scalar.copy(ar_in[:, 512:D], psum_r1[:])
                nc.scalar.copy(ar_in[:, D:D + 1], psum_s[:])
                cc_in = cc_pool.tile([1, AR_LEN], f32, name="cc_in")
                cc_out = cc_pool.tile([1, AR_LEN], f32, name="cc_out",
                                      addr_space="Shared")
                nc.gpsimd.dma_start(cc_in[:], ar_in[:])
                nc.gpsimd.collective_compute(
                    "AllReduce", ALU.add, replica_groups=rg,
                    ins=[cc_in[:].opt()], outs=[cc_out[:].opt()])
                ar_out = stepbuf.tile([1, AR_LEN], f32, name="ar_out")
                nc.gpsimd.dma_start(ar_out[:], cc_out[:])

                # ---- current = 0.8 * (r/s) + 0.2 * current ----
                inv_s = stepbuf.tile([1, 1], f32, name="inv_s")
                nc.vector.reciprocal(inv_s[:], ar_out[:, D:D + 1])
                sc08 = stepbuf.tile([1, 1], f32, name="sc08")
                nc.vector.tensor_scalar_mul(sc08[:], inv_s[:], 0.8)
                ret = stepbuf.tile([1, D], f32, name="ret")
                nc.vector.tensor_scalar(ret[:], ar_out[:, 0:D], sc08[:], None,
                                        ALU.mult)
                cur_next = stepbuf.tile([1, D], f32, name="cur_next")
                nc.vector.tensor_scalar_mul(cur_next[:], cur[:], 0.2)
                nc.vector.tensor_tensor(cur_next[:], cur_next[:], ret[:], ALU.add)
                cur = cur_next

            # ---- mismatch = mean((iso - cur)^2) ----
            diff = singles.tile([1, D], f32)
            nc.vector.tensor_tensor(diff[:], iso_sb[:], cur[:], ALU.subtract)
            dq2 = singles.tile([1, D], f32)
            mm = singles.tile([1, 1], f32)
            nc.vector._custom_dve(
                TENSOR_TENSOR_REDUCE, out=dq2[:], in0=diff[:], in1=diff[:],
                s0=0.0, s1=1.0 / D, accum_out=mm[:])

            nc.scalar.copy(out_sb[:, 0:D], cur[:])
            nc.scalar.copy(out_sb[:, D:D + 1], mm[:])
            nc.sync.dma_start(out_d[:], out_sb[:])

    nc.compile()
    return nc


_cache = {}


def _get_nc(n_shard):
    if n_shard not in _cache:
        _cache[n_shard] = build(n_shard)
    return _cache[n_shard]


def kernel(isocortex_state, ca3_matrix):
    iso = np.ascontiguousarray(np.asarray(isocortex_state, dtype=np.float32))
    ca3 = np.asarray(ca3_matrix, dtype=np.float32)
    n = ca3.shape[0]
    n_shard = n // N_CORES
    nc = _get_nc(n_shard)
    shards = ca3.reshape(N_CORES, n_shard, D)
    in_maps = [
        {"isocortex_state": iso, "ca3_matrix": np.ascontiguousarray(shards[i])}
        for i in range(N_CORES)
    ]
    r = run_bass_kernel_spmd(nc, in_maps, core_ids=list(range(N_CORES)))
    out = r.results[0]["out"]
    current = np.array(out[:, 0:D], dtype=np.float32)
    mismatch = np.array(out[0, D], dtype=np.float32)
    return current, mismatch


# revision 9
# speedup vs baseline: 1.0693x; 1.0104x over previous
"""Distributed Trainium2 kernel for the AllocortexSystem retrieval problem.

Reference semantics:
    sparse_prior = where(iso > 0.1, iso, 0)
    mem_norm = max(||row||, 1e-8) per ca3 row
    current = sparse_prior
    5x:
        q_norm = max(||current||, 1e-8)
        sim = (ca3 @ q) / (mem_norm * q_norm)       # cosine sims, in [-1, 1]
        w = softmax(sim)                             # global over all rows
        retrieved = w @ ca3
        current = 0.8 * retrieved + 0.2 * current
    mismatch = mean((iso - current)^2)

Strategy: shard ca3 row-wise over 8 cores. Since sims are cosine similarities
in [-1, 1], softmax needs no max subtraction: each step is ONE streaming pass
over the local shard computing s_partial = sum(exp(sim_i)) and
r_partial = sum(exp(sim_i) * row_i), followed by a tiny [1, 1032] AllReduce.

Per 128-row tile:
  - DVE custom TENSOR_TENSOR_REDUCE: fused (row * q_bcast) multiply +
    free-axis sum -> per-row dots (1/q_norm is pre-folded into q).
  - ACT Exp with per-partition scale = cached 1/mem_norm -> unnormalized w.
  - PE matmul (lhsT = w column [128,1], rhs = tile [128,512]x2) PSUM-accumulated
    across all tiles -> r_partial.
Pass 1 also computes row sumsq via ACT Square+accum_out in the same pass;
1/mem_norm comes from a DVE bitcast-Newton rsqrt (no ACT table switches:
only Exp/Square/Copy are used on ACT, all resident in one table set).
"""

import numpy as np

import concourse.bass as bass
import concourse.bacc as bacc
import concourse.tile as tile
from concourse import mybir
from concourse.bass_utils import run_bass_kernel_spmd
from concourse.dve_ops import TENSOR_TENSOR_REDUCE

f32 = mybir.dt.float32
bf16 = mybir.dt.bfloat16
i32 = mybir.dt.int32
AF = mybir.ActivationFunctionType
ALU = mybir.AluOpType

N_CORES = 8
N_FULL = 262144
D = 1024
P = 128
STEPS = 5
INV_EPS = 1e8          # 1 / EPS, clamp for rsqrt
THR = 0.1
AR_LEN = 1032          # 1024 (r partial) + 1 (s partial) + 7 pad
RSQRT_SEED = 0x5F3759DF


def _rsqrt(nc, y, t1, x, n_iter=3):
    """y = rsqrt(x) elementwise via bit-trick seed + Newton (DVE only).
    y, t1, x: same-shape fp32 APs (t1 scratch). Clamped to INV_EPS so the
    result equals 1/max(sqrt(x), eps) for all x >= 0."""
    nc.vector.tensor_scalar(y.bitcast(i32), x.bitcast(i32),
                            1, None, ALU.arith_shift_right)
    nc.vector.tensor_scalar(y.bitcast(i32), y.bitcast(i32),
                            -1, RSQRT_SEED, ALU.mult, ALU.add)
    for _ in range(n_iter):
        nc.vector.tensor_tensor(t1, y, y, ALU.mult)
        nc.vector.tensor_tensor(t1, t1, x, ALU.mult)
        nc.vector.tensor_scalar(t1, t1, -0.5, 1.5, ALU.mult, ALU.add)
        nc.vector.tensor_tensor(y, y, t1, ALU.mult)
    nc.vector.tensor_scalar_min(y, y, INV_EPS)


def build(n_shard=N_FULL // N_CORES, sup=4, sup_bufs=12):
    """Build + compile the SPMD program for one core's shard of n_shard rows."""
    T = n_shard // P            # 128-row tiles per shard
    nsup = T // sup             # supertiles (DMA granules) per pass
    assert nsup * sup == T

    nc = bacc.Bacc("TRN2", target_bir_lowering=False, debug=False,
                   num_devices=N_CORES)
    iso_d = nc.dram_tensor("isocortex_state", [1, D], f32, kind="ExternalInput")
    ca3_d = nc.dram_tensor("ca3_matrix", [n_shard, D], f32, kind="ExternalInput")
    out_d = nc.dram_tensor("out", [1, D + 1], f32, kind="ExternalOutput")

    rg = [list(range(N_CORES))]

    with tile.TileContext(nc) as tc:
        with (
            tc.tile_pool(name="sup", bufs=sup_bufs) as sup_pool,
            tc.tile_pool(name="singles", bufs=1) as singles,
            tc.tile_pool(name="dummies", bufs=2) as dummies,
            tc.tile_pool(name="stepbuf", bufs=2) as stepbuf,
            tc.tile_pool(name="psum_r", bufs=2, space="PSUM") as psum_r_pool,
            tc.tile_pool(name="psum_m", bufs=1, space="PSUM") as psum_m_pool,
            tc.tile_pool(name="cc", bufs=2, space="DRAM") as cc_pool,
        ):
            # ---- persistent buffers ----
            iso_sb = singles.tile([1, D], f32)
            ones_1x128 = singles.tile([1, P], f32)
            ones_128 = singles.tile([P, 1], f32)
            ones_8 = singles.tile([N_CORES, 1], f32)
            inv_norms = singles.tile([P, T], f32)   # 1/max(||row||,eps)
            ss = singles.tile([P, T], f32)          # row sumsq (pass 1)
            nrm_t1 = singles.tile([P, sup], f32)    # rsqrt scratch
            ar_in = singles.tile([1, AR_LEN], f32)
            out_sb = singles.tile([1, D + 1], f32)

            nc.sync.dma_start(iso_sb[:], iso_d[:])
            nc.vector.memset(ones_1x128[:], 1.0)
            nc.vector.memset(ones_128[:], 1.0)
            nc.vector.memset(ones_8[:], 1.0)
            nc.vector.memset(ar_in[:, D:AR_LEN], 0.0)

            # ---- q0 = sparse prior ----
            cur = singles.tile([1, D], f32, name="cur0")
            mask = singles.tile([1, D], f32)
            nc.vector.tensor_scalar(mask[:], iso_sb[:], THR, None, ALU.is_gt)
            nc.vector.tensor_tensor(cur[:], mask[:], iso_sb[:], ALU.mult)

            for step in range(STEPS):
                first = step == 0
                # ---- prologue: qn = cur / max(||cur||, eps), bcast to 128p
                dq = stepbuf.tile([1, D], f32, name="dq")
                ssq = stepbuf.tile([1, 1], f32, name="ssq")
                nc.vector._custom_dve(
                    TENSOR_TENSOR_REDUCE, out=dq[:], in0=cur[:], in1=cur[:],
                    s0=0.0, s1=1.0, accum_out=ssq[:])
                invq = stepbuf.tile([1, 1], f32, name="invq")
                sc_t1 = stepbuf.tile([1, 1], f32, name="sc_t1")
                _rsqrt(nc, invq[:], sc_t1[:], ssq[:])
                qn = stepbuf.tile([1, D], f32, name="qn")
                nc.vector.tensor_scalar(qn[:], cur[:], invq[:], None, ALU.mult)

                psum_b0 = psum_m_pool.tile([P, 512], f32, name="psum_b0")
                psum_b1 = psum_m_pool.tile([P, 512], f32, name="psum_b1")
                nc.tensor.matmul(psum_b0[:], ones_1x128[:], qn[:, 0:512],
                                 start=True, stop=True)
                nc.tensor.matmul(psum_b1[:], ones_1x128[:], qn[:, 512:D],
                                 start=True, stop=True)
                qn_b = stepbuf.tile([P, D], bf16, name="qn_b")
                nc.scalar.copy(qn_b[:, 0:512], psum_b0[:])
                nc.scalar.copy(qn_b[:, 512:D], psum_b1[:])

                # ---- single fused pass over the shard ----
                sims = stepbuf.tile([P, T], f32, name="sims")
                e_buf = stepbuf.tile([P, T], bf16, name="e_buf")
                psum_r0 = psum_r_pool.tile([1, 512], f32, name="psum_r0")
                psum_r1 = psum_r_pool.tile([1, 512], f32, name="psum_r1")

                for s in range(nsup):
                    st = sup_pool.tile([P, sup, D], bf16, name="st")
                    src = ca3_d[s * sup * P:(s + 1) * sup * P, :]
                    nc.gpsimd.dma_start(st[:], src.rearrange("(j p) d -> p j d", p=P))
                    cols = np.s_[:, s * sup:(s + 1) * sup]
                    for j in range(sup):
                        t = s * sup + j
                        ttro = dummies.tile([P, D], bf16, name="ttro")
                        if j == 0:
                            # 2x-mode multiply on DVE, reduce on (idle) ACT
                            nc.vector.tensor_tensor(ttro[:], st[:, j, :],
                                                    qn_b[:], ALU.mult)
                            nc.scalar.activation(ttro[:], ttro[:], AF.Copy,
                                                 accum_out=sims[:, t:t + 1])
                        else:
                            nc.vector._custom_dve(
                                TENSOR_TENSOR_REDUCE, out=ttro[:],
                                in0=st[:, j, :], in1=qn_b[:],
                                s0=0.0, s1=1.0, accum_out=sims[:, t:t + 1])
                        if first:
                            sqo = dummies.tile([P, D], bf16, name="sqo")
                            nc.scalar.activation(sqo[:], st[:, j, :], AF.Square,
                                                 accum_out=ss[:, t:t + 1])
                    if first:
                        _rsqrt(nc, inv_norms[cols], nrm_t1[:], ss[cols])
                    # batched scale + exp for the supertile's 4 sims columns
                    sims_sc = dummies.tile([P, sup], f32, name="sims_sc")
                    nc.vector.tensor_tensor(sims_sc[:], sims[cols],
                                            inv_norms[cols], ALU.mult)
                    nc.scalar.activation(e_buf[cols], sims_sc[:], AF.Exp)
                    for j in range(sup):
                        t = s * sup + j
                        nc.tensor.matmul(psum_r0[:], e_buf[:, t:t + 1],
                                         st[:, j, 0:512],
                                         start=(t == 0), stop=(t == T - 1))
                        nc.tensor.matmul(psum_r1[:], e_buf[:, t:t + 1],
                                         st[:, j, 512:D],
                                         start=(t == 0), stop=(t == T - 1))

                # ---- s_partial = sum(e) ----
                e_rowsum = stepbuf.tile([P, 1], f32, name="e_rowsum")
                nc.vector.tensor_reduce(e_rowsum[:], e_buf[:],
                                        mybir.AxisListType.X, ALU.add)
                psum_s = psum_m_pool.tile([1, 1], f32, name="psum_s")
                nc.tensor.matmul(psum_s[:], e_rowsum[:], ones_128[:],
                                 start=True, stop=True)

                # ---- AllReduce [r_partial | s_partial] ----
                nc.scalar.copy(ar_in[:, 0:512], psum_r0[:])
                nc.scalar.copy(ar_in[:, 512:D], psum_r1[:])
                nc.scalar.copy(ar_in[:, D:D + 1], psum_s[:])
                cc_in = cc_pool.tile([1, AR_LEN], f32, name="cc_in")
                cc_out = cc_pool.tile([N_CORES, AR_LEN], f32, name="cc_out",
                                      addr_space="Shared")
                nc.gpsimd.dma_start(cc_in[:], ar_in[:])
                nc.gpsimd.collective_compute(
                    "AllGather", ALU.bypass, replica_groups=rg,
                    ins=[cc_in[:].opt()], outs=[cc_out[:].opt()])
                ag_sb = stepbuf.tile([N_CORES, AR_LEN], f32, name="ag_sb")
                nc.gpsimd.dma_start(ag_sb[:], cc_out[:])
                psum_ag0 = psum_m_pool.tile([1, 512], f32, name="psum_ag0",
                                            tag="psum_b0")
                psum_ag1 = psum_m_pool.tile([1, 512], f32, name="psum_ag1",
                                            tag="psum_b1")
                psum_ags = psum_m_pool.tile([1, 1], f32, name="psum_ags",
                                            tag="psum_s")
                nc.tensor.matmul(psum_ag0[:], ones_8[:], ag_sb[:, 0:512],
                                 start=True, stop=True)
                nc.tensor.matmul(psum_ag1[:], ones_8[:], ag_sb[:, 512:D],
                                 start=True, stop=True)
                nc.tensor.matmul(psum_ags[:], ones_8[:], ag_sb[:, D:D + 1],
                                 start=True, stop=True)
                ar_out = stepbuf.tile([1, AR_LEN], f32, name="ar_out")
                nc.scalar.copy(ar_out[:, 0:512], psum_ag0[:])
                nc.scalar.copy(ar_out[:, 512:D], psum_ag1[:])
                nc.scalar.copy(ar_out[:, D:D + 1], psum_ags[:])

                # ---- current = 0.8 * (r/s) + 0.2 * current ----
                inv_s = stepbuf.tile([1, 1], f32, name="inv_s")
                nc.vector.reciprocal(inv_s[:], ar_out[:, D:D + 1])
                sc08 = stepbuf.tile([1, 1], f32, name="sc08")
                nc.vector.tensor_scalar_mul(sc08[:], inv_s[:], 0.8)
                ret = stepbuf.tile([1, D], f32, name="ret")
                nc.vector.tensor_scalar(ret[:], ar_out[:, 0:D], sc08[:], None,
                                        ALU.mult)
                cur_next = stepbuf.tile([1, D], f32, name="cur_next")
                nc.vector.tensor_scalar_mul(cur_next[:], cur[:], 0.2)
                nc.vector.tensor_tensor(cur_next[:], cur_next[:], ret[:], ALU.add)
                cur = cur_next

            # ---- mismatch = mean((iso - cur)^2) ----
            diff = singles.tile([1, D], f32)
            nc.vector.tensor_tensor(diff[:], iso_sb[:], cur[:], ALU.subtract)
            dq2 = singles.tile([1, D], f32)
            mm = singles.tile([1, 1], f32)
            nc.vector._custom_dve(
                TENSOR_TENSOR_REDUCE, out=dq2[:], in0=diff[:], in1=diff[:],
                s0=0.0, s1=1.0 / D, accum_out=mm[:])

            nc.scalar.copy(out_sb[:, 0:D], cur[:])
            nc.scalar.copy(out_sb[:, D:D + 1], mm[:])
            nc.sync.dma_start(out_d[:], out_sb[:])

    nc.compile()
    return nc


_cache = {}


def _get_nc(n_shard):
    if n_shard not in _cache:
        _cache[n_shard] = build(n_shard)
    return _cache[n_shard]


def kernel(isocortex_state, ca3_matrix):
    iso = np.ascontiguousarray(np.asarray(isocortex_state, dtype=np.float32))
    ca3 = np.asarray(ca3_matrix, dtype=np.float32)
    n = ca3.shape[0]
    n_shard = n // N_CORES
    nc = _get_nc(n_shard)
    shards = ca3.reshape(N_CORES, n_shard, D)
    in_maps = [
        {"isocortex_state": iso, "ca3_matrix": np.ascontiguousarray(shards[i])}
        for i in range(N_CORES)
    ]
    r = run_bass_kernel_spmd(nc, in_maps, core_ids=list(range(N_CORES)))
    out = r.results[0]["out"]
    current = np.array(out[:, 0:D], dtype=np.float32)
    mismatch = np.array(out[0, D], dtype=np.float32)
    return current, mismatch


# revision 10
# speedup vs baseline: 1.2213x; 1.1422x over previous
"""Distributed Trainium2 kernel for the AllocortexSystem retrieval problem.

Reference semantics:
    sparse_prior = where(iso > 0.1, iso, 0)
    mem_norm = max(||row||, 1e-8) per ca3 row
    current = sparse_prior
    5x:
        q_norm = max(||current||, 1e-8)
        sim = (ca3 @ q) / (mem_norm * q_norm)       # cosine sims, in [-1, 1]
        w = softmax(sim)                             # global over all rows
        retrieved = w @ ca3
        current = 0.8 * retrieved + 0.2 * current
    mismatch = mean((iso - current)^2)

Strategy: shard ca3 row-wise over 8 cores. Since sims are cosine similarities
in [-1, 1], softmax needs no max subtraction: each step is ONE streaming pass
over the local shard computing s_partial = sum(exp(sim_i)) and
r_partial = sum(exp(sim_i) * row_i), followed by a tiny [1, 1032] AllReduce.

Per 128-row tile:
  - DVE custom TENSOR_TENSOR_REDUCE: fused (row * q_bcast) multiply +
    free-axis sum -> per-row dots (1/q_norm is pre-folded into q).
  - ACT Exp with per-partition scale = cached 1/mem_norm -> unnormalized w.
  - PE matmul (lhsT = w column [128,1], rhs = tile [128,512]x2) PSUM-accumulated
    across all tiles -> r_partial.
Pass 1 also computes row sumsq via ACT Square+accum_out in the same pass;
1/mem_norm comes from a DVE bitcast-Newton rsqrt (no ACT table switches:
only Exp/Square/Copy are used on ACT, all resident in one table set).
"""

import numpy as np

import concourse.bass as bass
import concourse.bacc as bacc
import concourse.tile as tile
from concourse import mybir
from concourse.bass_utils import run_bass_kernel_spmd
from concourse.dve_ops import TENSOR_TENSOR_REDUCE

f32 = mybir.dt.float32
bf16 = mybir.dt.bfloat16
i32 = mybir.dt.int32
AF = mybir.ActivationFunctionType
ALU = mybir.AluOpType

N_CORES = 8
N_FULL = 262144
D = 1024
P = 128
STEPS = 5
INV_EPS = 1e8          # 1 / EPS, clamp for rsqrt
THR = 0.1
AR_LEN = 1032          # 1024 (r partial) + 1 (s partial) + 7 pad
RSQRT_SEED = 0x5F3759DF


def _rsqrt(nc, y, t1, x, n_iter=3):
    """y = rsqrt(x) elementwise via bit-trick seed + Newton (DVE only).
    y, t1, x: same-shape fp32 APs (t1 scratch). Clamped to INV_EPS so the
    result equals 1/max(sqrt(x), eps) for all x >= 0."""
    nc.vector.tensor_scalar(y.bitcast(i32), x.bitcast(i32),
                            1, None, ALU.arith_shift_right)
    nc.vector.tensor_scalar(y.bitcast(i32), y.bitcast(i32),
                            -1, RSQRT_SEED, ALU.mult, ALU.add)
    for _ in range(n_iter):
        nc.vector.tensor_tensor(t1, y, y, ALU.mult)
        nc.vector.tensor_tensor(t1, t1, x, ALU.mult)
        nc.vector.tensor_scalar(t1, t1, -0.5, 1.5, ALU.mult, ALU.add)
        nc.vector.tensor_tensor(y, y, t1, ALU.mult)
    nc.vector.tensor_scalar_min(y, y, INV_EPS)


def build(n_shard=N_FULL // N_CORES, sup=8, sup_bufs=6):
    """Build + compile the SPMD program for one core's shard of n_shard rows."""
    T = n_shard // P            # 128-row tiles per shard
    nsup = T // sup             # supertiles (DMA granules) per pass
    assert nsup * sup == T

    nc = bacc.Bacc("TRN2", target_bir_lowering=False, debug=False,
                   num_devices=N_CORES)
    iso_d = nc.dram_tensor("isocortex_state", [1, D], f32, kind="ExternalInput")
    ca3_d = nc.dram_tensor("ca3_matrix", [n_shard, D], f32, kind="ExternalInput")
    out_d = nc.dram_tensor("out", [1, D + 1], f32, kind="ExternalOutput")

    rg = [list(range(N_CORES))]

    with tile.TileContext(nc) as tc:
        with (
            tc.tile_pool(name="sup", bufs=sup_bufs) as sup_pool,
            tc.tile_pool(name="singles", bufs=1) as singles,
            tc.tile_pool(name="dummies", bufs=2) as dummies,
            tc.tile_pool(name="stepbuf", bufs=2) as stepbuf,
            tc.tile_pool(name="psum_r", bufs=2, space="PSUM") as psum_r_pool,
            tc.tile_pool(name="psum_m", bufs=1, space="PSUM") as psum_m_pool,
            tc.tile_pool(name="cc", bufs=2, space="DRAM") as cc_pool,
        ):
            # ---- persistent buffers ----
            iso_sb = singles.tile([1, D], f32)
            ones_1x128 = singles.tile([1, P], f32)
            ones_128 = singles.tile([P, 1], f32)
            ones_8 = singles.tile([N_CORES, 1], f32)
            inv_norms = singles.tile([P, T], f32)   # 1/max(||row||,eps)
            ss = singles.tile([P, T], f32)          # row sumsq (pass 1)
            nrm_t1 = singles.tile([P, sup], f32)    # rsqrt scratch
            ar_in = singles.tile([1, AR_LEN], f32)
            out_sb = singles.tile([1, D + 1], f32)

            nc.sync.dma_start(iso_sb[:], iso_d[:])
            nc.vector.memset(ones_1x128[:], 1.0)
            nc.vector.memset(ones_128[:], 1.0)
            nc.vector.memset(ones_8[:], 1.0)
            nc.vector.memset(ar_in[:, D:AR_LEN], 0.0)

            # ---- q0 = sparse prior ----
            cur = singles.tile([1, D], f32, name="cur0")
            mask = singles.tile([1, D], f32)
            nc.vector.tensor_scalar(mask[:], iso_sb[:], THR, None, ALU.is_gt)
            nc.vector.tensor_tensor(cur[:], mask[:], iso_sb[:], ALU.mult)

            for step in range(STEPS):
                first = step == 0
                # ---- prologue: qn = cur / max(||cur||, eps), bcast to 128p
                dq = stepbuf.tile([1, D], f32, name="dq")
                ssq = stepbuf.tile([1, 1], f32, name="ssq")
                nc.vector._custom_dve(
                    TENSOR_TENSOR_REDUCE, out=dq[:], in0=cur[:], in1=cur[:],
                    s0=0.0, s1=1.0, accum_out=ssq[:])
                invq = stepbuf.tile([1, 1], f32, name="invq")
                sc_t1 = stepbuf.tile([1, 1], f32, name="sc_t1")
                _rsqrt(nc, invq[:], sc_t1[:], ssq[:])
                qn = stepbuf.tile([1, D], f32, name="qn")
                nc.vector.tensor_scalar(qn[:], cur[:], invq[:], None, ALU.mult)

                psum_b0 = psum_m_pool.tile([P, 512], f32, name="psum_b0")
                psum_b1 = psum_m_pool.tile([P, 512], f32, name="psum_b1")
                nc.tensor.matmul(psum_b0[:], ones_1x128[:], qn[:, 0:512],
                                 start=True, stop=True)
                nc.tensor.matmul(psum_b1[:], ones_1x128[:], qn[:, 512:D],
                                 start=True, stop=True)
                qn_b = stepbuf.tile([P, D], bf16, name="qn_b")
                nc.scalar.copy(qn_b[:, 0:512], psum_b0[:])
                nc.scalar.copy(qn_b[:, 512:D], psum_b1[:])

                # ---- single fused pass over the shard ----
                sims = stepbuf.tile([P, T], f32, name="sims")
                e_buf = stepbuf.tile([P, T], bf16, name="e_buf")
                psum_r0 = psum_r_pool.tile([1, 512], f32, name="psum_r0")
                psum_r1 = psum_r_pool.tile([1, 512], f32, name="psum_r1")

                for s in range(nsup):
                    st = sup_pool.tile([P, sup, D], bf16, name="st")
                    src = ca3_d[s * sup * P:(s + 1) * sup * P, :]
                    nc.gpsimd.dma_start(st[:], src.rearrange("(p j) d -> p j d", p=P))
                    cols = np.s_[:, s * sup:(s + 1) * sup]
                    for j in range(sup):
                        t = s * sup + j
                        ttro = dummies.tile([P, D], bf16, name="ttro")
                        if j < (1 if first else 2):
                            # 2x-mode multiply on DVE, reduce on (idle) ACT
                            nc.vector.tensor_tensor(ttro[:], st[:, j, :],
                                                    qn_b[:], ALU.mult)
                            nc.scalar.activation(ttro[:], ttro[:], AF.Copy,
                                                 accum_out=sims[:, t:t + 1])
                        else:
                            nc.vector._custom_dve(
                                TENSOR_TENSOR_REDUCE, out=ttro[:],
                                in0=st[:, j, :], in1=qn_b[:],
                                s0=0.0, s1=1.0, accum_out=sims[:, t:t + 1])
                        if first:
                            sqo = dummies.tile([P, D], bf16, name="sqo")
                            nc.scalar.activation(sqo[:], st[:, j, :], AF.Square,
                                                 accum_out=ss[:, t:t + 1])
                    if first:
                        _rsqrt(nc, inv_norms[cols], nrm_t1[:], ss[cols])
                    # batched scale + exp for the supertile's 4 sims columns
                    sims_sc = dummies.tile([P, sup], f32, name="sims_sc")
                    nc.vector.tensor_tensor(sims_sc[:], sims[cols],
                                            inv_norms[cols], ALU.mult)
                    nc.scalar.activation(e_buf[cols], sims_sc[:], AF.Exp)
                    for j in range(sup):
                        t = s * sup + j
                        nc.tensor.matmul(psum_r0[:], e_buf[:, t:t + 1],
                                         st[:, j, 0:512],
                                         start=(t == 0), stop=(t == T - 1))
                        nc.tensor.matmul(psum_r1[:], e_buf[:, t:t + 1],
                                         st[:, j, 512:D],
                                         start=(t == 0), stop=(t == T - 1))

                # ---- s_partial = sum(e) ----
                e_rowsum = stepbuf.tile([P, 1], f32, name="e_rowsum")
                nc.vector.tensor_reduce(e_rowsum[:], e_buf[:],
                                        mybir.AxisListType.X, ALU.add)
                psum_s = psum_m_pool.tile([1, 1], f32, name="psum_s")
                nc.tensor.matmul(psum_s[:], e_rowsum[:], ones_128[:],
                                 start=True, stop=True)

                # ---- AllReduce [r_partial | s_partial] ----
                nc.scalar.copy(ar_in[:, 0:512], psum_r0[:])
                nc.scalar.copy(ar_in[:, 512:D], psum_r1[:])
                nc.scalar.copy(ar_in[:, D:D + 1], psum_s[:])
                cc_in = cc_pool.tile([1, AR_LEN], f32, name="cc_in")
                cc_out = cc_pool.tile([N_CORES, AR_LEN], f32, name="cc_out",
                                      addr_space="Shared")
                nc.sync.dma_start(cc_in[:], ar_in[:])
                nc.gpsimd.collective_compute(
                    "AllGather", ALU.bypass, replica_groups=rg,
                    ins=[cc_in[:].opt()], outs=[cc_out[:].opt()])
                ag_sb = stepbuf.tile([N_CORES, AR_LEN], f32, name="ag_sb")
                nc.sync.dma_start(ag_sb[:], cc_out[:])
                psum_ag0 = psum_m_pool.tile([1, 512], f32, name="psum_ag0",
                                            tag="psum_b0")
                psum_ag1 = psum_m_pool.tile([1, 512], f32, name="psum_ag1",
                                            tag="psum_b1")
                psum_ags = psum_m_pool.tile([1, 1], f32, name="psum_ags",
                                            tag="psum_s")
                nc.tensor.matmul(psum_ag0[:], ones_8[:], ag_sb[:, 0:512],
                                 start=True, stop=True)
                nc.tensor.matmul(psum_ag1[:], ones_8[:], ag_sb[:, 512:D],
                                 start=True, stop=True)
                nc.tensor.matmul(psum_ags[:], ones_8[:], ag_sb[:, D:D + 1],
                                 start=True, stop=True)
                ar_out = stepbuf.tile([1, AR_LEN], f32, name="ar_out")
                nc.scalar.copy(ar_out[:, 0:512], psum_ag0[:])
                nc.scalar.copy(ar_out[:, 512:D], psum_ag1[:])
                nc.scalar.copy(ar_out[:, D:D + 1], psum_ags[:])

                # ---- current = 0.8 * (r/s) + 0.2 * current ----
                inv_s = stepbuf.tile([1, 1], f32, name="inv_s")
                nc.vector.reciprocal(inv_s[:], ar_out[:, D:D + 1])
                sc08 = stepbuf.tile([1, 1], f32, name="sc08")
                nc.vector.tensor_scalar_mul(sc08[:], inv_s[:], 0.8)
                ret = stepbuf.tile([1, D], f32, name="ret")
                nc.vector.tensor_scalar(ret[:], ar_out[:, 0:D], sc08[:], None,
                                        ALU.mult)
                cur_next = stepbuf.tile([1, D], f32, name="cur_next")
                nc.vector.tensor_scalar_mul(cur_next[:], cur[:], 0.2)
                nc.vector.tensor_tensor(cur_next[:], cur_next[:], ret[:], ALU.add)
                cur = cur_next

            # ---- mismatch = mean((iso - cur)^2) ----
            diff = singles.tile([1, D], f32)
            nc.vector.tensor_tensor(diff[:], iso_sb[:], cur[:], ALU.subtract)
            dq2 = singles.tile([1, D], f32)
            mm = singles.tile([1, 1], f32)
            nc.vector._custom_dve(
                TENSOR_TENSOR_REDUCE, out=dq2[:], in0=diff[:], in1=diff[:],
                s0=0.0, s1=1.0 / D, accum_out=mm[:])

            nc.scalar.copy(out_sb[:, 0:D], cur[:])
            nc.scalar.copy(out_sb[:, D:D + 1], mm[:])
            nc.sync.dma_start(out_d[:], out_sb[:])

    nc.compile()
    return nc


_cache = {}


def _get_nc(n_shard):
    if n_shard not in _cache:
        _cache[n_shard] = build(n_shard)
    return _cache[n_shard]


def kernel(isocortex_state, ca3_matrix):
    iso = np.ascontiguousarray(np.asarray(isocortex_state, dtype=np.float32))
    ca3 = np.asarray(ca3_matrix, dtype=np.float32)
    n = ca3.shape[0]
    n_shard = n // N_CORES
    nc = _get_nc(n_shard)
    shards = ca3.reshape(N_CORES, n_shard, D)
    in_maps = [
        {"isocortex_state": iso, "ca3_matrix": np.ascontiguousarray(shards[i])}
        for i in range(N_CORES)
    ]
    r = run_bass_kernel_spmd(nc, in_maps, core_ids=list(range(N_CORES)))
    out = r.results[0]["out"]
    current = np.array(out[:, 0:D], dtype=np.float32)
    mismatch = np.array(out[0, D], dtype=np.float32)
    return current, mismatch


# revision 22
# speedup vs baseline: 1.2437x; 1.0183x over previous
"""Distributed Trainium2 kernel for the AllocortexSystem retrieval problem.

Reference semantics:
    sparse_prior = where(iso > 0.1, iso, 0)
    mem_norm = max(||row||, 1e-8) per ca3 row
    current = sparse_prior
    5x:
        q_norm = max(||current||, 1e-8)
        sim = (ca3 @ q) / (mem_norm * q_norm)       # cosine sims, in [-1, 1]
        w = softmax(sim)                             # global over all rows
        retrieved = w @ ca3
        current = 0.8 * retrieved + 0.2 * current
    mismatch = mean((iso - current)^2)

Strategy: shard ca3 row-wise over 8 cores. Since sims are cosine similarities
in [-1, 1], softmax needs no max subtraction: each step is ONE streaming pass
over the local shard computing s_partial = sum(exp(sim_i)) and
r_partial = sum(exp(sim_i) * row_i), followed by a tiny [1, 1032] AllReduce.

Per 128-row tile:
  - DVE custom TENSOR_TENSOR_REDUCE: fused (row * q_bcast) multiply +
    free-axis sum -> per-row dots (1/q_norm is pre-folded into q).
  - ACT Exp with per-partition scale = cached 1/mem_norm -> unnormalized w.
  - PE matmul (lhsT = w column [128,1], rhs = tile [128,512]x2) PSUM-accumulated
    across all tiles -> r_partial.
Pass 1 also computes row sumsq via ACT Square+accum_out in the same pass;
1/mem_norm comes from a DVE bitcast-Newton rsqrt (no ACT table switches:
only Exp/Square/Copy are used on ACT, all resident in one table set).
"""

import numpy as np

import concourse.bass as bass
import concourse.bacc as bacc
import concourse.tile as tile
from concourse import mybir
from concourse.bass_utils import run_bass_kernel_spmd
from concourse.dve_ops import TENSOR_TENSOR_REDUCE

f32 = mybir.dt.float32
bf16 = mybir.dt.bfloat16
i32 = mybir.dt.int32
AF = mybir.ActivationFunctionType
ALU = mybir.AluOpType

N_CORES = 8
N_FULL = 262144
D = 1024
P = 128
STEPS = 5
INV_EPS = 1e8          # 1 / EPS, clamp for rsqrt
THR = 0.1
AR_LEN = 1032          # 1024 (r partial) + 1 (s partial) + 7 pad
RSQRT_SEED = 0x5F3759DF


def _rsqrt(nc, y, t1, x, n_iter=3):
    """y = rsqrt(x) elementwise via bit-trick seed + Newton (DVE only).
    y, t1, x: same-shape fp32 APs (t1 scratch). Clamped to INV_EPS so the
    result equals 1/max(sqrt(x), eps) for all x >= 0."""
    nc.vector.tensor_scalar(y.bitcast(i32), x.bitcast(i32),
                            1, None, ALU.arith_shift_right)
    nc.vector.tensor_scalar(y.bitcast(i32), y.bitcast(i32),
                            -1, RSQRT_SEED, ALU.mult, ALU.add)
    for _ in range(n_iter):
        nc.vector.tensor_tensor(t1, y, y, ALU.mult)
        nc.vector.tensor_tensor(t1, t1, x, ALU.mult)
        nc.vector.tensor_scalar(t1, t1, -0.5, 1.5, ALU.mult, ALU.add)
        nc.vector.tensor_tensor(y, y, t1, ALU.mult)
    nc.vector.tensor_scalar_min(y, y, INV_EPS)


def build(n_shard=N_FULL // N_CORES, sup=8, sup_bufs=7):
    """Build + compile the SPMD program for one core's shard of n_shard rows."""
    T = n_shard // P            # 128-row tiles per shard
    sup = min(sup, T)
    nsup = T // sup             # supertiles (DMA granules) per pass
    assert nsup * sup == T

    nc = bacc.Bacc("TRN2", target_bir_lowering=False, debug=False,
                   num_devices=N_CORES)
    iso_d = nc.dram_tensor("isocortex_state", [1, D], f32, kind="ExternalInput")
    ca3_d = nc.dram_tensor("ca3_matrix", [n_shard, D], f32, kind="ExternalInput")
    out_d = nc.dram_tensor("out", [1, D + 1], f32, kind="ExternalOutput")

    rg = [list(range(N_CORES))]

    with tile.TileContext(nc) as tc:
        with (
            tc.tile_pool(name="sup", bufs=sup_bufs) as sup_pool,
            tc.tile_pool(name="singles", bufs=1) as singles,
            tc.tile_pool(name="dummies", bufs=2) as dummies,
            tc.tile_pool(name="stepbuf", bufs=2) as stepbuf,
            tc.tile_pool(name="psum_r", bufs=2, space="PSUM") as psum_r_pool,
            tc.tile_pool(name="psum_m", bufs=1, space="PSUM") as psum_m_pool,
            tc.tile_pool(name="cc", bufs=2, space="DRAM") as cc_pool,
        ):
            # ---- persistent buffers ----
            iso_sb = singles.tile([1, D], f32)
            ones_1x128 = singles.tile([1, P], f32)
            ones_128 = singles.tile([P, 1], f32)
            ones_8 = singles.tile([N_CORES, 1], f32)
            inv_norms = singles.tile([P, T], f32)   # 1/max(||row||,eps)
            ss = singles.tile([P, T], f32)          # row sumsq (pass 1)
            nrm_t1 = singles.tile([P, sup], f32)    # rsqrt scratch
            ar_in = singles.tile([1, AR_LEN], f32)
            out_sb = singles.tile([1, D + 1], f32)

            nc.sync.dma_start(iso_sb[:], iso_d[:])
            nc.vector.memset(ones_1x128[:], 1.0)
            nc.vector.memset(ones_128[:], 1.0)
            nc.vector.memset(ones_8[:], 1.0)
            nc.vector.memset(ar_in[:, D:AR_LEN], 0.0)

            # ---- q0 = sparse prior ----
            cur = singles.tile([1, D], f32, name="cur0")
            mask = singles.tile([1, D], f32)
            nc.vector.tensor_scalar(mask[:], iso_sb[:], THR, None, ALU.is_gt)
            nc.vector.tensor_tensor(cur[:], mask[:], iso_sb[:], ALU.mult)

            for step in range(STEPS):
                first = step == 0
                # ---- prologue: qn = cur / max(||cur||, eps), bcast to 128p
                dq = stepbuf.tile([1, D], f32, name="dq")
                ssq = stepbuf.tile([1, 1], f32, name="ssq")
                nc.vector._custom_dve(
                    TENSOR_TENSOR_REDUCE, out=dq[:], in0=cur[:], in1=cur[:],
                    s0=0.0, s1=1.0, accum_out=ssq[:])
                invq = stepbuf.tile([1, 1], f32, name="invq")
                sc_t1 = stepbuf.tile([1, 1], f32, name="sc_t1")
                _rsqrt(nc, invq[:], sc_t1[:], ssq[:])
                qn = stepbuf.tile([1, D], f32, name="qn")
                nc.vector.tensor_scalar(qn[:], cur[:], invq[:], None, ALU.mult)

                psum_b0 = psum_m_pool.tile([P, 512], f32, name="psum_b0")
                psum_b1 = psum_m_pool.tile([P, 512], f32, name="psum_b1")
                nc.tensor.matmul(psum_b0[:], ones_1x128[:], qn[:, 0:512],
                                 start=True, stop=True)
                nc.tensor.matmul(psum_b1[:], ones_1x128[:], qn[:, 512:D],
                                 start=True, stop=True)
                qn_b = stepbuf.tile([P, D], bf16, name="qn_b")
                nc.scalar.copy(qn_b[:, 0:512], psum_b0[:])
                nc.scalar.copy(qn_b[:, 512:D], psum_b1[:])

                # ---- single fused pass over the shard ----
                sims = stepbuf.tile([P, T], f32, name="sims")
                e_buf = stepbuf.tile([P, T], bf16, name="e_buf")
                psum_r0 = psum_r_pool.tile([1, 512], f32, name="psum_r0")
                psum_r1 = psum_r_pool.tile([1, 512], f32, name="psum_r1")

                for s in range(nsup):
                    st = sup_pool.tile([P, sup, D], bf16, name="st")
                    src = ca3_d[s * sup * P:(s + 1) * sup * P, :]
                    nc.gpsimd.dma_start(st[:], src.rearrange("(p j) d -> p j d", p=P))
                    cols = np.s_[:, s * sup:(s + 1) * sup]
                    for j in range(sup):
                        t = s * sup + j
                        ttro = dummies.tile([P, D], bf16, name="ttro")
                        if j < (1 if first else 2):
                            # 2x-mode multiply on DVE, reduce on (idle) ACT
                            nc.vector.tensor_tensor(ttro[:], st[:, j, :],
                                                    qn_b[:], ALU.mult)
                            nc.scalar.activation(ttro[:], ttro[:], AF.Copy,
                                                 accum_out=sims[:, t:t + 1])
                        else:
                            nc.vector._custom_dve(
                                TENSOR_TENSOR_REDUCE, out=ttro[:],
                                in0=st[:, j, :], in1=qn_b[:],
                                s0=0.0, s1=1.0, accum_out=sims[:, t:t + 1])
                        if first:
                            sqo = dummies.tile([P, D], bf16, name="sqo")
                            nc.scalar.activation(sqo[:], st[:, j, :], AF.Square,
                                                 accum_out=ss[:, t:t + 1])
                    if first:
                        _rsqrt(nc, inv_norms[cols], nrm_t1[:], ss[cols])
                    # batched scale + exp for the supertile's 4 sims columns
                    sims_sc = dummies.tile([P, sup], f32, name="sims_sc")
                    nc.vector.tensor_tensor(sims_sc[:], sims[cols],
                                            inv_norms[cols], ALU.mult)
                    nc.scalar.activation(e_buf[cols], sims_sc[:], AF.Exp)
                    for j in range(sup):
                        t = s * sup + j
                        nc.tensor.matmul(psum_r0[:], e_buf[:, t:t + 1],
                                         st[:, j, 0:512],
                                         start=(t == 0), stop=(t == T - 1))
                        nc.tensor.matmul(psum_r1[:], e_buf[:, t:t + 1],
                                         st[:, j, 512:D],
                                         start=(t == 0), stop=(t == T - 1))

                # ---- s_partial = sum(e) ----
                e_rowsum = stepbuf.tile([P, 1], f32, name="e_rowsum")
                nc.vector.tensor_reduce(e_rowsum[:], e_buf[:],
                                        mybir.AxisListType.X, ALU.add)
                psum_s = psum_m_pool.tile([1, 1], f32, name="psum_s")
                nc.tensor.matmul(psum_s[:], e_rowsum[:], ones_128[:],
                                 start=True, stop=True)

                # ---- AllReduce [r_partial | s_partial] ----
                nc.scalar.copy(ar_in[:, 0:512], psum_r0[:])
                nc.scalar.copy(ar_in[:, 512:D], psum_r1[:])
                nc.scalar.copy(ar_in[:, D:D + 1], psum_s[:])
                cc_in = cc_pool.tile([1, AR_LEN], f32, name="cc_in")
                cc_out = cc_pool.tile([N_CORES, AR_LEN], f32, name="cc_out",
                                      addr_space="Shared")
                nc.sync.dma_start(cc_in[:], ar_in[:])
                nc.gpsimd.collective_compute(
                    "AllGather", ALU.bypass, replica_groups=rg,
                    ins=[cc_in[:].opt()], outs=[cc_out[:].opt()])
                ag_sb = stepbuf.tile([N_CORES, AR_LEN], f32, name="ag_sb")
                nc.sync.dma_start(ag_sb[:], cc_out[:])
                psum_ag0 = psum_m_pool.tile([1, 512], f32, name="psum_ag0",
                                            tag="psum_b0")
                psum_ag1 = psum_m_pool.tile([1, 512], f32, name="psum_ag1",
                                            tag="psum_b1")
                psum_ags = psum_m_pool.tile([1, 1], f32, name="psum_ags",
                                            tag="psum_s")
                nc.tensor.matmul(psum_ag0[:], ones_8[:], ag_sb[:, 0:512],
                                 start=True, stop=True)
                nc.tensor.matmul(psum_ag1[:], ones_8[:], ag_sb[:, 512:D],
                                 start=True, stop=True)
                nc.tensor.matmul(psum_ags[:], ones_8[:], ag_sb[:, D:D + 1],
                                 start=True, stop=True)
                ar_out = stepbuf.tile([1, AR_LEN], f32, name="ar_out")
                nc.scalar.copy(ar_out[:, 0:512], psum_ag0[:])
                nc.scalar.copy(ar_out[:, 512:D], psum_ag1[:])
                nc.scalar.copy(ar_out[:, D:D + 1], psum_ags[:])

                # ---- current = 0.8 * (r/s) + 0.2 * current ----
                inv_s = stepbuf.tile([1, 1], f32, name="inv_s")
                nc.vector.reciprocal(inv_s[:], ar_out[:, D:D + 1])
                sc08 = stepbuf.tile([1, 1], f32, name="sc08")
                nc.vector.tensor_scalar_mul(sc08[:], inv_s[:], 0.8)
                ret = stepbuf.tile([1, D], f32, name="ret")
                nc.vector.tensor_scalar(ret[:], ar_out[:, 0:D], sc08[:], None,
                                        ALU.mult)
                cur_next = stepbuf.tile([1, D], f32, name="cur_next")
                nc.vector.tensor_scalar_mul(cur_next[:], cur[:], 0.2)
                nc.vector.tensor_tensor(cur_next[:], cur_next[:], ret[:], ALU.add)
                cur = cur_next

            # ---- mismatch = mean((iso - cur)^2) ----
            diff = singles.tile([1, D], f32)
            nc.vector.tensor_tensor(diff[:], iso_sb[:], cur[:], ALU.subtract)
            dq2 = singles.tile([1, D], f32)
            mm = singles.tile([1, 1], f32)
            nc.vector._custom_dve(
                TENSOR_TENSOR_REDUCE, out=dq2[:], in0=diff[:], in1=diff[:],
                s0=0.0, s1=1.0 / D, accum_out=mm[:])

            nc.scalar.copy(out_sb[:, 0:D], cur[:])
            nc.scalar.copy(out_sb[:, D:D + 1], mm[:])
            nc.sync.dma_start(out_d[:], out_sb[:])

    nc.compile()
    return nc


_cache = {}


def _get_nc(n_shard):
    if n_shard not in _cache:
        _cache[n_shard] = build(n_shard)
    return _cache[n_shard]


def kernel(isocortex_state, ca3_matrix):
    iso = np.ascontiguousarray(np.asarray(isocortex_state, dtype=np.float32))
    ca3 = np.asarray(ca3_matrix, dtype=np.float32)
    n = ca3.shape[0]
    n_shard = n // N_CORES
    nc = _get_nc(n_shard)
    shards = ca3.reshape(N_CORES, n_shard, D)
    in_maps = [
        {"isocortex_state": iso, "ca3_matrix": np.ascontiguousarray(shards[i])}
        for i in range(N_CORES)
    ]
    r = run_bass_kernel_spmd(nc, in_maps, core_ids=list(range(N_CORES)))
    out = r.results[0]["out"]
    current = np.array(out[:, 0:D], dtype=np.float32)
    mismatch = np.array(out[0, D], dtype=np.float32)
    return current, mismatch
